# revision 44
# baseline (speedup 1.0000x reference)
import sys

if "/opt/trn_rl_repo" not in sys.path:
    sys.path.insert(0, "/opt/trn_rl_repo")

import numpy as np
from ml_dtypes import bfloat16 as np_bf16
from ml_dtypes import float8_e4m3fn as np_f8
import concourse.bacc as bacc
import concourse.bass as bass
import concourse.mybir as mybir
import concourse.tile as tile
from concourse.bass_utils import run_bass_kernel_spmd
from concourse.masks import make_identity

# Problem dims (hardcoded per spec)
DIM = 2048
DMEDIA = 1024
HEADS = 16
DH = 64
INNER = 1024
FF = 8192
LAT = 64
B = 4
NTOK = 2048
T = 1024          # tokens per core (one batch element, half its tokens)
P = 128
EPS = 1e-5
NCORES = 8

DC = DIM // P      # 16 dim chunks
IC = INNER // P    # 8 inner chunks
MC = DMEDIA // P   # 8 media-dim chunks
FC = FF // P       # 64 ffn chunks
TS = T // P        # 8 token sub-tiles
SCALE = DH ** -0.5

F32 = mybir.dt.float32
BF16 = mybir.dt.bfloat16
F8 = mybir.dt.float8e4
W8SCALE = 32.0
AF = mybir.ActivationFunctionType


def build_program():
    nc = bacc.Bacc("TRN2", target_bir_lowering=False, debug=False)

    x_d = nc.dram_tensor("x", [T, DIM], F32, kind="ExternalInput")
    media_d = nc.dram_tensor("media", [LAT, DMEDIA], F32, kind="ExternalInput")
    masklog_d = nc.dram_tensor("masklog", [LAT, 1], F32, kind="ExternalInput")
    wq_d = nc.dram_tensor("Wq", [DIM, INNER], BF16, kind="ExternalInput")
    wk_d = nc.dram_tensor("Wk", [IC * P, MC * P], BF16, kind="ExternalInput")
    wv_d = nc.dram_tensor("Wv", [2 * P, MC * 512], BF16, kind="ExternalInput")
    wo_d = nc.dram_tensor("Wo", [P, IC * DIM], BF16, kind="ExternalInput")
    w1_d = nc.dram_tensor("W1", [FC * P, 2048], F8, kind="ExternalInput")
    w2_d = nc.dram_tensor("W2", [128 * P, 1024], F8, kind="ExternalInput")
    g1s_d = nc.dram_tensor("g1s", [P, DC], F32, kind="ExternalInput")
    b1s_d = nc.dram_tensor("b1s", [P, DC], F32, kind="ExternalInput")
    g2_d = nc.dram_tensor("g2", [P, DC], F32, kind="ExternalInput")
    b2_d = nc.dram_tensor("b2", [P, DC], F32, kind="ExternalInput")
    tg1_d = nc.dram_tensor("tg1", [1, 1], F32, kind="ExternalInput")  # tanh(attn_gate)
    tg2_d = nc.dram_tensor("tg2", [1, 1], F32, kind="ExternalInput")  # tanh(ff_gate)
    sumsel_d = nc.dram_tensor("sumsel", [P, 2], BF16, kind="ExternalInput")
    onehot_d = nc.dram_tensor("onehot", [2, P], BF16, kind="ExternalInput")
    out_d = nc.dram_tensor("out", [T, DIM], F32, kind="ExternalOutput")

    from contextlib import ExitStack

    with tile.TileContext(nc) as tc, ExitStack() as es_pp:
        pp = es_pp.enter_context(tc.tile_pool(name="persist", bufs=1))
        ident = pp.tile([P, P], F32)
        make_identity(nc, ident)
        eps_sb = pp.tile([P, 1], F32)
        nc.vector.memset(eps_sb, EPS)
        tg1_sb = pp.tile([P, 1], F32)
        tg2_sb = pp.tile([P, 1], F32)
        nc.sync.dma_start(tg1_sb[:], bass.AP(
            tensor=tg1_d.ap().tensor, offset=0, ap=[[0, P], [1, 1]]))
        nc.sync.dma_start(tg2_sb[:], bass.AP(
            tensor=tg2_d.ap().tensor, offset=0, ap=[[0, P], [1, 1]]))
        mask_sb = pp.tile([P, 1], F32)  # masklog replicated on both halves
        nc.sync.dma_start(mask_sb[0:LAT, :], masklog_d[:])
        nc.sync.dma_start(mask_sb[LAT:P, :], masklog_d[:])

        # ln gains/biases as [P, DC]: element (p, c) = g[c*128+p]
        # (attention scale already folded into g1s/b1s on host)
        g1s_sb = pp.tile([P, DC], F32)
        b1s_sb = pp.tile([P, DC], F32)
        g2_sb = pp.tile([P, DC], F32)
        b2_sb = pp.tile([P, DC], F32)
        nc.sync.dma_start(g1s_sb[:], g1s_d[:])
        nc.sync.dma_start(b1s_sb[:], b1s_d[:])
        nc.sync.dma_start(g2_sb[:], g2_d[:])
        nc.sync.dma_start(b2_sb[:], b2_d[:])

        # col0: ones on partitions 0-63, col1: ones on partitions 64-127
        sumsel2 = pp.tile([P, 2], BF16)
        nc.sync.dma_start(sumsel2[:], sumsel_d[:])
        # row0 -> broadcast into cols 0-63, row1 -> cols 64-127
        onehot2 = pp.tile([2, P], BF16)
        nc.sync.dma_start(onehot2[:], onehot_d[:])
        onehot2f = pp.tile([2, P], F32)  # f32 twin for f32-moving matmuls
        nc.vector.tensor_copy(onehot2f[:], onehot2[:])

        # ---- Long-lived pools (properly nested open/close) -------------
        es_qn2 = ExitStack()    # qn2T8: EF..end of FFN2
        qn2Tp = es_qn2.enter_context(tc.tile_pool(name="qn2T_pool", bufs=1))
        qn2T8 = [qn2Tp.tile([P, 2, T], F8, tag=f"qn2T8_{i}",
                            name=f"qn2T8_{i}")
                 for i in range(DC // 2)]
        # FFN1-backfill pools, opened before the front so FFN1's first-
        # half matmuls can fill PE idle gaps during the second token half's
        # LN/attention chain (their SBUF/PSUM never overlaps front pools).
        es_h1a = ExitStack()    # h1 quarter: th0 tokens, f 0..31
        h1ap = es_h1a.enter_context(tc.tile_pool(name="h1a_pool", bufs=1))
        h1b0a = h1ap.tile([P, 16, 2, 512], F8)
        es_w1g = ExitStack()
        w1g = es_w1g.enter_context(tc.tile_pool(name="w1g", bufs=3))
        es_psg = ExitStack()
        psg = es_psg.enter_context(
            tc.tile_pool(name="ps_g", bufs=2, space="PSUM"))
        es_kv = ExitStack()     # kT/v2: phase A..end of attention
        kvp = es_kv.enter_context(tc.tile_pool(name="kv_pool", bufs=1))

        # ---------------- Phase A: media^T, K/V projections -------------
        kT_sb = kvp.tile([P, IC, LAT], BF16)            # k^T [inner, lat]
        # v packed per parity: v2_sb[(h%2)*64+lat, 4*(h//8)+(h%8)//2, dh]
        v2_sb = kvp.tile([P, IC, DH], BF16)

        with tc.tile_pool(name="ps_a", bufs=2, space="PSUM") as ps_a, \
             tc.tile_pool(name="media_p", bufs=1) as mp, \
             tc.tile_pool(name="wk_st", bufs=4) as wk_st, \
             tc.tile_pool(name="wv_st", bufs=2) as wv_st:
            media_sb = mp.tile([LAT, DMEDIA], F32)
            nc.scalar.dma_start(media_sb[:], media_d[:])
            mediaT = mp.tile([P, MC, LAT], BF16)   # media^T (bf16 for matmul)
            for c in range(MC):
                pt = ps_a.tile([P, 512], F32, tag="psa")
                nc.tensor.transpose(
                    pt[:, :LAT], media_sb[:, c * P : (c + 1) * P],
                    ident[:LAT, :LAT])
                nc.vector.tensor_copy(mediaT[:, c, :], pt[:, :LAT])
            # k^T: per inner chunk, accumulate over media-dim chunks
            for ic in range(IC):
                wk = wk_st.tile([P, MC * P], BF16, tag="wk")
                nc.scalar.dma_start(wk[:], wk_d[ic * P : (ic + 1) * P, :])
                pk = ps_a.tile([P, 512], F32, tag="psa")
                for mc in range(MC):
                    nc.tensor.matmul(
                        pk[:, :LAT], wk[:, mc * P : (mc + 1) * P],
                        mediaT[:, mc, :],
                        start=(mc == 0), stop=(mc == MC - 1))
                nc.vector.tensor_copy(kT_sb[:, ic, :], pk[:, :LAT])
            # v packed by parity: heads h%2==parity at partition base
            # parity*64, slot 4*half+g where h = 8*half + 2g + parity
            for half in range(2):
                wv = wv_st.tile([P, MC, 512], BF16, tag="wv")
                nc.scalar.dma_start(
                    wv[:],
                    wv_d[half * P : (half + 1) * P, :].rearrange(
                        "p (mc i) -> p mc i", i=512))
                for parity in range(2):
                    po = parity * LAT
                    pv = ps_a.tile([P, 512], F32, tag="psa")
                    for g in range(4):
                        for mc in range(MC):
                            nc.tensor.matmul(
                                pv[po : po + LAT, g * DH : (g + 1) * DH],
                                mediaT[:, mc, :],
                                wv[:, mc,
                                   g * 2 * DH + parity * DH :
                                   g * 2 * DH + (parity + 1) * DH],
                                start=(mc == 0), stop=(mc == MC - 1))
                    nc.vector.tensor_copy(
                        v2_sb[po : po + LAT, 4 * half : 4 * half + 4, :],
                        pv[po : po + LAT, :256].rearrange(
                            "l (g q) -> l g q", q=DH))

        # ---- Front pipeline pools (persist across both token halves) --
        es_wo = ExitStack()
        wop = es_wo.enter_context(tc.tile_pool(name="wo_pool", bufs=1))
        wo_sb = wop.tile([P, IC, DIM], BF16, tag="wo")
        es_ao = ExitStack()
        aop = es_ao.enter_context(tc.tile_pool(name="ao_pool", bufs=1))
        es_qa = ExitStack()
        qap = es_qa.enter_context(tc.tile_pool(name="qa_pool", bufs=1))
        es_qnT = ExitStack()
        qnTp = es_qnT.enter_context(tc.tile_pool(name="qnT_pool", bufs=1))
        es_bw = ExitStack()     # B-phase working pools: outer so the second
        # half's LN work overlaps the first half's attention with no
        # pool-release dependencies
        xlp = es_bw.enter_context(tc.tile_pool(name="xload", bufs=3))
        qntp = es_bw.enter_context(tc.tile_pool(name="qn_t", bufs=2))
        stp = es_bw.enter_context(tc.tile_pool(name="stats", bufs=8))
        ps_tr = es_bw.enter_context(
            tc.tile_pool(name="ps_tr", bufs=2, space="PSUM"))

        HD = DIM // 2
        for th in range(2):
            # ---- B(th): LN1 + transpose -> qnT[c] (bf16, 512 tokens) ----
            qnT = [qnTp.tile([P, 512], BF16, tag=f"qnT{c}",
                             name=f"qnT{c}_{th}") for c in range(DC)]
            for gg in range(2):
                grp = th * 2 + gg
                qts = []
                for i2 in range(2):
                    i = grp * 2 + i2
                    xhs = []
                    st = stp.tile([P, 4, 6], F32, tag="st")
                    for hf in range(2):
                        xh = xlp.tile([P, HD], F32, tag="x")
                        nc.sync.dma_start(
                            xh[:], x_d[i * P : (i + 1) * P,
                                       hf * HD : (hf + 1) * HD])
                        for j in range(2):
                            nc.vector.bn_stats(
                                st[:, 2 * hf + j, :],
                                xh[:, j * 512 : (j + 1) * 512])
                        xhs.append(xh)
                    mv = stp.tile([P, 2], F32, tag="mv")
                    nc.vector.bn_aggr(mv[:], st[:])
                    rstd = stp.tile([P, 1], F32, tag="rstd")
                    nc.scalar.activation(
                        rstd[:], mv[:, 1:2], AF.Sqrt, bias=eps_sb[:])
                    nc.vector.reciprocal(rstd[:], rstd[:])
                    # center+scale on ACT: (x-mu)*rstd = rstd*x + (-mu*rstd)
                    nmr = stp.tile([P, 1], F32, tag="nmr")
                    nc.vector.tensor_mul(nmr[:], mv[:, 0:1], rstd[:])
                    nc.vector.tensor_scalar_mul(nmr[:], nmr[:], -1.0)
                    qt = qntp.tile([P, DIM], F32, tag="qn")
                    for hf in range(2):
                        nc.scalar.activation(
                            qt[:, hf * HD : (hf + 1) * HD], xhs[hf][:],
                            AF.Identity, bias=nmr[:], scale=rstd[:])
                    qts.append(qt)
                for c in range(DC):
                    pt = ps_tr.tile([P, 256], F32, tag="tr")
                    for i2 in range(2):
                        nc.tensor.transpose(
                            pt[:, i2 * P : (i2 + 1) * P],
                            qts[i2][:, c * P : (c + 1) * P], ident[:])
                    if c % 2 == 0:
                        nc.scalar.activation(
                            qnT[c][:, gg * 256 : (gg + 1) * 256], pt[:],
                            AF.Identity, bias=b1s_sb[:, c : c + 1],
                            scale=g1s_sb[:, c : c + 1])
                    else:
                        nc.vector.tensor_scalar(
                            qnT[c][:, gg * 256 : (gg + 1) * 256], pt[:],
                            scalar1=g1s_sb[:, c : c + 1],
                            scalar2=b1s_sb[:, c : c + 1],
                            op0=mybir.AluOpType.mult, op1=mybir.AluOpType.add)

            # ---- C(th): Q projection (two 4-bank passes) ----------------
            qT = [qap.tile([P, 512], BF16, tag=f"qT{i}", name=f"qT{i}_{th}")
                  for i in range(IC)]
            attnT = [qap.tile([P, 512], BF16, tag=f"aT{i}",
                              name=f"attnT{i}_{th}") for i in range(IC)]
            with tc.tile_pool(name="wq_st", bufs=4) as wqst, \
                 tc.tile_pool(name="ps_q", bufs=4, space="PSUM") as ps_q:
                for icg in range(2):
                    pqs = [ps_q.tile([P, 512], F32, tag="q", name=f"pq{i}")
                           for i in range(4)]
                    for dc in range(DC):
                        wqt = wqst.tile([P, 512], BF16, tag="wq")
                        nc.scalar.dma_start(
                            wqt[:], wq_d[dc * P : (dc + 1) * P,
                                         icg * 512 : (icg + 1) * 512])
                        for i4 in range(4):
                            nc.tensor.matmul(
                                pqs[i4], wqt[:, i4 * P : (i4 + 1) * P],
                                qnT[dc][:],
                                start=(dc == 0), stop=(dc == DC - 1))
                    for i4 in range(4):
                        ic = icg * 4 + i4
                        if ic % 2 == 0:
                            nc.scalar.copy(qT[ic][:], pqs[i4])
                        else:
                            nc.vector.tensor_copy(qT[ic][:], pqs[i4])

            if th == 0:
                # Wo load on the ACT HWDGE ring, overlapping attention
                nc.scalar.dma_start(
                    wo_sb[:], wo_d[:].rearrange("p (ic d) -> p ic d", d=DIM))

            # ---- D(th): attention --------------------------------------
            attn_oT = [aop.tile([P, 512], BF16, tag=f"ao{i}",
                                name=f"attn_oT{i}_{th}") for i in range(IC)]
            with tc.tile_pool(name="ps_at", bufs=3, space="PSUM") as ps_at:
                for ic in range(IC):
                    ps = ps_at.tile([P, 512], F32, tag="at")
                    for parity in range(2):
                        po = parity * LAT
                        nc.tensor.matmul(
                            ps[po : po + LAT, :],
                            kT_sb[po : po + LAT, ic, :],
                            qT[ic][po : po + LAT, :],
                            start=True, stop=True)
                    # exp(sim + masklog) fused on ACT
                    nc.scalar.activation(
                        attnT[ic][:], ps[:], AF.Exp, bias=mask_sb[:])

            # softmax denominators via ACT ln/exp; AV runs on the
            # UNNORMALIZED attnT and 1/sum is applied at PSUM evacuation
            with tc.tile_pool(name="ps_s2", bufs=1, space="PSUM") as ps_s2, \
                 tc.tile_pool(name="ps_b", bufs=1, space="PSUM") as ps_b, \
                 tc.tile_pool(name="ps_av", bufs=2, space="PSUM") as ps_av, \
                 tc.tile_pool(name="rp_pool", bufs=3) as rpp:
                for ic in range(IC):
                    # rows 0/1 = sumexp of heads 2ic / 2ic+1
                    ps2 = ps_s2.tile([2, 512], F32, tag="s2")
                    nc.tensor.matmul(
                        ps2[:], sumsel2[:], attnT[ic][:],
                        start=True, stop=True)
                    s2ln = rpp.tile([2, 512], F32, tag="s2ln")
                    nc.scalar.activation(s2ln[:], ps2[:], AF.Ln)
                    pb = ps_b.tile([P, 512], F32, tag="b")
                    nc.tensor.matmul(
                        pb[:], onehot2f[:], s2ln[:], start=True, stop=True)
                    rp = rpp.tile([P, 512], BF16, tag="rp")
                    with nc.allow_low_precision(reason="softmax denom bf16"):
                        nc.scalar.activation(rp[:], pb[:], AF.Exp, scale=-1.0)
                    pav = ps_av.tile([P, 512], F32, tag="av")
                    for hh in range(2):
                        h = ic * 2 + hh
                        po = hh * LAT
                        vslot = 4 * (h // 8) + (h % 8) // 2
                        nc.tensor.matmul(
                            pav[po : po + LAT, :],
                            v2_sb[po : po + LAT, vslot, :],
                            attnT[ic][po : po + LAT, :],
                            start=True, stop=True)
                    nc.vector.tensor_mul(attn_oT[ic][:], pav[:], rp[:])

            # ---- EF(th): O-proj + residual, LN2, -> qn2T8, x1 -> out_d --
            with tc.tile_pool(name="xstr", bufs=3) as xstr, \
                 tc.tile_pool(name="x1t", bufs=2) as x1p, \
                 tc.tile_pool(name="qn2_t", bufs=2) as qn2tp, \
                 tc.tile_pool(name="stats2", bufs=8) as st2p, \
                 tc.tile_pool(name="ps_o", bufs=4, space="PSUM") as ps_o:
                for gg in range(2):
                    grp = th * 2 + gg
                    q2ts = []
                    for t2 in range(2):
                        ts_ = grp * 2 + t2
                        x1t = x1p.tile([P, DIM], F32, tag="x1")
                        for dc4 in range(4):
                            sl = slice(dc4 * 512, (dc4 + 1) * 512)
                            po_ = ps_o.tile([P, 512], F32, tag="o")
                            for ic in range(IC):
                                nc.tensor.matmul(
                                    po_[:],
                                    attn_oT[ic][:, (ts_ % 4) * P :
                                                 (ts_ % 4 + 1) * P],
                                    wo_sb[:, ic, sl],
                                    start=(ic == 0), stop=(ic == IC - 1))
                            nc.scalar.mul(x1t[:, sl], po_[:], tg1_sb[:])
                            xc = xstr.tile([P, 512], F32, tag="xc")
                            nc.sync.dma_start(
                                xc[:], x_d[ts_ * P : (ts_ + 1) * P, sl])
                            nc.vector.tensor_add(
                                x1t[:, sl], x1t[:, sl], xc[:])
                        # LN2 stats + center
                        st = st2p.tile([P, 4, 6], F32, tag="st2")
                        for j in range(4):
                            nc.vector.bn_stats(
                                st[:, j, :], x1t[:, j * 512 : (j + 1) * 512])
                        mv = st2p.tile([P, 2], F32, tag="mv2")
                        nc.vector.bn_aggr(mv[:], st[:])
                        rstd = st2p.tile([P, 1], F32, tag="rstd2")
                        nc.scalar.activation(
                            rstd[:], mv[:, 1:2], AF.Sqrt, bias=eps_sb[:])
                        nc.vector.reciprocal(rstd[:], rstd[:])
                        nmr = st2p.tile([P, 1], F32, tag="nmr2")
                        nc.vector.tensor_mul(nmr[:], mv[:, 0:1], rstd[:])
                        nc.vector.tensor_scalar_mul(nmr[:], nmr[:], -1.0)
                        q2t = qn2tp.tile([P, DIM], F32, tag="qn2")
                        nc.scalar.activation(
                            q2t[:], x1t[:], AF.Identity,
                            bias=nmr[:], scale=rstd[:])
                        q2ts.append(q2t)
                        nc.sync.dma_start(
                            out_d[ts_ * P : (ts_ + 1) * P, :], x1t[:])
                    for c in range(DC):
                        pt = ps_tr.tile([P, 256], F32, tag="tr")
                        for t2 in range(2):
                            nc.tensor.transpose(
                                pt[:, t2 * P : (t2 + 1) * P],
                                q2ts[t2][:, c * P : (c + 1) * P], ident[:])
                        dst = qn2T8[c // 2][:, c % 2,
                                           grp * 256 : (grp + 1) * 256]
                        with nc.allow_low_precision(reason="fp8 ffn inputs"):
                            if c % 2 == 0:
                                nc.scalar.activation(
                                    dst, pt[:],
                                    AF.Identity, bias=b2_sb[:, c : c + 1],
                                    scale=g2_sb[:, c : c + 1])
                            else:
                                nc.vector.tensor_scalar(
                                    dst, pt[:],
                                    scalar1=g2_sb[:, c : c + 1],
                                    scalar2=b2_sb[:, c : c + 1],
                                    op0=mybir.AluOpType.mult,
                                    op1=mybir.AluOpType.add)

        es_bw.close()
        es_qnT.close()
        es_qa.close()
        es_ao.close()
        es_wo.close()
        es_kv.close()

        # ---- Phase G: FFN1 (fp8 DoubleRow), three passes ----------------
        # h1[p, g8, r, t] = gelu(h1) at ffn-row g8*256 + r*128 + p, stored
        # as three tiles: (th0, f<32) -> h1b0a [early pool, backfills the
        # front], (th0, f>=32) -> h1b0b, (th1, all f) -> h1b1.
        def g_pass(th, f0, f1, dst):
            for f in range(f0, f1):
                w1t = w1g.tile([P, DC // 2, 2, P], F8, tag="w1",
                               name=f"w1t_{th}_{f}")
                nc.sync.dma_start(
                    w1t[:],
                    w1_d[f * P : (f + 1) * P, :].rearrange(
                        "p (c8 r q) -> p c8 r q", r=2, q=P))
                pg = psg.tile([P, 512], F32, tag="g", name=f"pg_{th}_{f}")
                for c8 in range(DC // 2):
                    nc.tensor.matmul(
                        pg[:], w1t[:, c8, :, :],
                        qn2T8[c8][:, :, th * 512 : (th + 1) * 512],
                        start=(c8 == 0), stop=(c8 == DC // 2 - 1),
                        perf_mode=mybir.MatmulPerfMode.DoubleRow)
                with nc.allow_low_precision(reason="fp8 ffn h1"):
                    nc.scalar.activation(
                        dst[:, (f - f0) // 2, f % 2, :],
                        pg[:], AF.Gelu, scale=1.0 / W8SCALE)

        g_pass(0, 0, 32, h1b0a)
        es_h1b0b = ExitStack()
        h1bp = es_h1b0b.enter_context(tc.tile_pool(name="h1b_pool", bufs=1))
        h1b0b = h1bp.tile([P, 16, 2, 512], F8)
        g_pass(0, 32, FC, h1b0b)
        es_h1b1 = ExitStack()
        h1cp = es_h1b1.enter_context(tc.tile_pool(name="h1c_pool", bufs=1))
        h1b1 = h1cp.tile([P, 32, 2, 512], F8)
        g_pass(1, 0, FC, h1b1)

        # ---- Phase H: FFN2 + gated residual accumulate ------------------
        with tc.tile_pool(name="w2_st", bufs=40) as w2st, \
             tc.tile_pool(name="outst", bufs=4) as outp, \
             tc.tile_pool(name="ps_f2", bufs=4, space="PSUM") as ps_f2:
            for dc4 in range(4):
                sl = slice(dc4 * 512, (dc4 + 1) * 512)
                w2ts = []
                for g8 in range(32):
                    w2t = w2st.tile([P, 2, 512], F8, tag="w2",
                                    name=f"w2t_{dc4}_{g8}")
                    nc.sync.dma_start(
                        w2t[:],
                        w2_d[(dc4 * 32 + g8) * P : (dc4 * 32 + g8 + 1) * P, :]
                        .rearrange("p (r d) -> p r d", d=512))
                    w2ts.append(w2t)
                for ts_ in range(TS):
                    th = ts_ // 4
                    tl = ts_ % 4
                    pos = ps_f2.tile([P, 512], F32, tag="f2")
                    for g8 in range(32):
                        if th == 0:
                            h1src = h1b0a if g8 < 16 else h1b0b
                            gl = g8 % 16
                        else:
                            h1src = h1b1
                            gl = g8
                        nc.tensor.matmul(
                            pos[:],
                            h1src[:, gl, :, tl * P : (tl + 1) * P],
                            w2ts[g8][:],
                            start=(g8 == 0), stop=(g8 == 31),
                            perf_mode=mybir.MatmulPerfMode.DoubleRow)
                    ot = outp.tile([P, 512], F32, tag="out")
                    if ts_ % 2 == 0:
                        nc.scalar.mul(ot[:], pos[:], tg2_sb[:])
                    else:
                        nc.vector.tensor_scalar(
                            ot[:], pos[:], scalar1=tg2_sb[:],
                            scalar2=None, op0=mybir.AluOpType.mult)
                    nc.gpsimd.dma_start(
                        out_d[ts_ * P : (ts_ + 1) * P, sl], ot[:],
                        accum_op=mybir.AluOpType.add)
        es_h1b1.close()
        es_h1b0b.close()
        es_psg.close()
        es_w1g.close()
        es_h1a.close()
        es_qn2.close()

    nc.compile()
    return nc


_CACHED = None


def _get_program():
    global _CACHED
    if _CACHED is None:
        _CACHED = build_program()
    return _CACHED


def _prep_weights(inputs):
    wq = np.asarray(inputs["Wq"], dtype=np.float32)
    wkv = np.asarray(inputs["Wkv"], dtype=np.float32)
    wo = np.asarray(inputs["Wo"], dtype=np.float32)
    w1 = np.asarray(inputs["W1"], dtype=np.float32)
    w2 = np.asarray(inputs["W2"], dtype=np.float32)
    g1 = np.asarray(inputs["ln_q_g"], dtype=np.float32)
    b1 = np.asarray(inputs["ln_q_b"], dtype=np.float32)
    g2 = np.asarray(inputs["ln_ff_g"], dtype=np.float32)
    b2 = np.asarray(inputs["ln_ff_b"], dtype=np.float32)

    wq_bf = wq.astype(np_bf16)                       # [DIM, INNER]
    wkv3 = wkv.reshape(MC, P, 2 * INNER)
    wk_prep = (wkv3[:, :, :INNER].reshape(MC, P, IC, P)
               .transpose(2, 1, 0, 3).astype(np_bf16).reshape(IC * P, MC * P))
    wv_prep = (wkv3[:, :, INNER:].reshape(MC, P, 2, 512)
               .transpose(2, 1, 0, 3).astype(np_bf16).reshape(2 * P, MC * 512))
    wo_prep = (wo.reshape(IC, P, DIM).transpose(1, 0, 2)
               .astype(np_bf16).reshape(P, IC * DIM))
    # w1_prep[f*128+p, c8*256+r*128+q] = 32*W1[c8*256+r*128+p, f*128+q]
    w1_prep = ((w1 * W8SCALE).reshape(8, 2, P, FC, P)
               .transpose(3, 2, 0, 1, 4).astype(np_f8).reshape(FC * P, 2048))
    # w2_prep[(dc4*32+g8)*128+p, r*512+d] = 32*W2[g8*256+r*128+p, dc4*512+d]
    w2_prep = ((w2 * W8SCALE).reshape(32, 2, P, 4, 512)
               .transpose(3, 0, 2, 1, 4).astype(np_f8).reshape(128 * P, 1024))
    g1s = np.ascontiguousarray((g1 * SCALE).reshape(DC, P).T)
    b1s = np.ascontiguousarray((b1 * SCALE).reshape(DC, P).T)
    g2p = np.ascontiguousarray(g2.reshape(DC, P).T)
    b2p = np.ascontiguousarray(b2.reshape(DC, P).T)
    tg1 = np.tanh(np.asarray(inputs["attn_gate"], dtype=np.float32)).reshape(1, 1)
    tg2 = (np.tanh(np.asarray(inputs["ff_gate"], dtype=np.float32))
           / W8SCALE).reshape(1, 1)
    sumsel_np = np.zeros((P, 2), dtype=np_bf16)
    sumsel_np[:LAT, 0] = 1.0
    sumsel_np[LAT:, 1] = 1.0
    onehot_np = np.ascontiguousarray(sumsel_np.T)
    return {
        "Wq": wq_bf, "Wk": wk_prep, "Wv": wv_prep, "Wo": wo_prep,
        "W1": w1_prep, "W2": w2_prep,
        "g1s": g1s, "b1s": b1s, "g2": g2p, "b2": b2p,
        "tg1": tg1, "tg2": tg2,
        "sumsel": sumsel_np, "onehot": onehot_np,
    }


def kernel(**inputs):
    x = np.asarray(inputs["x"], dtype=np.float32)
    media = np.asarray(inputs["media"], dtype=np.float32)
    mask = np.asarray(inputs["media_mask"])
    shared = _prep_weights(inputs)

    nc = _get_program()
    in_maps = []
    for core in range(NCORES):
        b = core // 2
        half = core % 2
        masklog = np.where(mask[b], 0.0, -50.0).astype(np.float32).reshape(LAT, 1)
        in_maps.append({
            "x": np.ascontiguousarray(x[b, half * T : (half + 1) * T, :]),
            "media": np.ascontiguousarray(media[b]),
            "masklog": masklog,
            **shared,
        })
    res = run_bass_kernel_spmd(nc, in_maps, core_ids=list(range(NCORES)))
    out = np.empty((B, NTOK, DIM), dtype=np.float32)
    for core in range(NCORES):
        b = core // 2
        half = core % 2
        out[b, half * T : (half + 1) * T, :] = res.results[core]["out"]
    return out
